# revision 5
# baseline (speedup 1.0000x reference)
# Additive (Bahdanau) attention kernel for Trainium2, data-parallel over batch
# on 8 NeuronCores.
#
# Per core (2 batches):
#   qp = query @ Wq            [256q, 128h]   (kept transposed: qpT [h, q])
#   kp = key @ Wk              [256k, 128h]   (kept natural:    kp  [k, h])
#   score[k, q] = sum_h v[h] * tanh(qp[q, h] + kp[k, h])
#   attention = softmax_q(score);  context = attention @ value
#
# The [k, q] plane for each h is built on the TensorEngine as two matmuls into
# PSUM: an identity pass that broadcasts kp[:, h] along q, plus an indicator
# pass that broadcasts qpT[h, :] along k. ScalarE applies tanh (fp16 out), and
# VectorE folds planes into the score accumulator with one fused
# (T * v_h) + acc op per plane. Softmax runs per k-tile with Exp+accum_out
# (scores are bounded by sum|v| <= 6.5, so no max subtraction is needed).

import numpy as np

B_TOTAL = 16
N_CORES = 8
B_LOC = B_TOTAL // N_CORES
L = 256      # l_q == l_k
D = 256      # q_dim == k_dim
H = 128      # attention dim
VD = 128     # value dim
P = 128      # partitions

_cache = {}


def _build():
    from contextlib import ExitStack

    import concourse.bacc as bacc
    import concourse.mybir as mybir
    import concourse.tile as tile
    
    f32 = mybir.dt.float32
    f16 = mybir.dt.float16
    AF = mybir.ActivationFunctionType
    OP = mybir.AluOpType

    nc = bacc.Bacc("TRN2", target_bir_lowering=False, debug=False)

    q_d = nc.dram_tensor("query", [B_LOC, L, D], f32, kind="ExternalInput")
    k_d = nc.dram_tensor("key", [B_LOC, L, D], f32, kind="ExternalInput")
    val_d = nc.dram_tensor("value", [B_LOC, L, VD], f32, kind="ExternalInput")
    wq_d = nc.dram_tensor("Wq", [D, H], f32, kind="ExternalInput")
    wk_d = nc.dram_tensor("Wk", [D, H], f32, kind="ExternalInput")
    vv_d = nc.dram_tensor("v", [H], f32, kind="ExternalInput")
    ctx_d = nc.dram_tensor("context", [B_LOC, L, VD], f32, kind="ExternalOutput")
    attn_d = nc.dram_tensor("attention", [B_LOC, L, L], f32, kind="ExternalOutput")

    KT = L // P   # k tiles (2)
    QT = L // P   # q tiles (2)
    DC = D // P   # d chunks (2)
    HH = 2        # h planes per PSUM tile (psum tile = HH banks)

    with tile.TileContext(nc) as tc, ExitStack() as ctx:
        singles = ctx.enter_context(tc.tile_pool(name="singles", bufs=1))
        io = ctx.enter_context(tc.tile_pool(name="io", bufs=2))
        tr = ctx.enter_context(tc.tile_pool(name="tr", bufs=2))
        proj = ctx.enter_context(tc.tile_pool(name="proj", bufs=2))
        tanhp = ctx.enter_context(tc.tile_pool(name="tanhp", bufs=3))
        accp = ctx.enter_context(tc.tile_pool(name="accp", bufs=2))
        soft = ctx.enter_context(tc.tile_pool(name="soft", bufs=2))
        outp = ctx.enter_context(tc.tile_pool(name="outp", bufs=2))
        psum_big = ctx.enter_context(tc.tile_pool(name="psum_big", bufs=3, space="PSUM"))
        psum_small = ctx.enter_context(tc.tile_pool(name="psum_small", bufs=2, space="PSUM"))

        # --- constants -----------------------------------------------------
        # delta[p, m] = p - m; one-hot identities via is_equal (much faster
        # than gpsimd affine_select for the big T128 constant below).
        delta = singles.tile([P, P], mybir.dt.int32, tag="delta")
        nc.gpsimd.iota(delta, pattern=[[-1, P]], base=0, channel_multiplier=1)
        ident32 = singles.tile([P, P], f32, tag="ident32")
        nc.vector.tensor_scalar(
            out=ident32, in0=delta, scalar1=0, scalar2=None, op0=OP.is_equal
        )
        ident16 = singles.tile([P, P], f16, tag="ident16")
        nc.vector.tensor_copy(out=ident16, in_=ident32)

        # T128[p, k, h] = 1.0 iff p == h. lhsT slice T128[:, :, h] is a K=128
        # stationary whose row h is all-ones: the matmul broadcasts moving
        # row h (a qpT row) to all 128 output partitions while keeping the
        # full PE array engaged (K=32 variants starve the HAM clock governor).
        t128 = singles.tile([P, P, P], f16, tag="t128")
        nc.vector.tensor_copy(
            out=t128, in_=ident16.unsqueeze(1).broadcast_to([P, P, P])
        )

        vb = singles.tile([P, H], f32, tag="vb")  # v broadcast across partitions
        nc.sync.dma_start(out=vb, in_=vv_d[:].unsqueeze(0).broadcast_to([P, H]))

        wq_sb = singles.tile([P, DC, H], f32, tag="wq")
        nc.sync.dma_start(out=wq_sb, in_=wq_d[:].rearrange("(c p) h -> p c h", p=P))
        wk_sb = singles.tile([P, DC, H], f32, tag="wk")
        nc.sync.dma_start(out=wk_sb, in_=wk_d[:].rearrange("(c p) h -> p c h", p=P))

        for b in range(B_LOC):
            # --- load ------------------------------------------------------
            q_sb = io.tile([P, QT, D], f32, tag="q_sb")
            nc.sync.dma_start(out=q_sb, in_=q_d[b].rearrange("(t p) d -> p t d", p=P))
            k_sb = io.tile([P, KT, D], f32, tag="k_sb")
            nc.sync.dma_start(out=k_sb, in_=k_d[b].rearrange("(t p) d -> p t d", p=P))
            val_sb = io.tile([P, QT, VD], f32, tag="val_sb")
            nc.sync.dma_start(out=val_sb, in_=val_d[b].rearrange("(t p) v -> p t v", p=P))

            # --- transpose query/key (PE transpose, fp32) ------------------
            qT_sb = tr.tile([P, DC, L], f32, tag="qT")
            kT_sb = tr.tile([P, DC, L], f32, tag="kT")
            for src, dst in ((q_sb, qT_sb), (k_sb, kT_sb)):
                for t in range(QT):
                    for dc in range(DC):
                        psT = psum_small.tile([P, 256], f32, tag="small")
                        nc.tensor.transpose(
                            psT[:, :P], src[:, t, dc * P:(dc + 1) * P], ident32
                        )
                        nc.vector.tensor_copy(
                            out=dst[:, dc, t * P:(t + 1) * P], in_=psT[:, :P]
                        )

            # --- projections ----------------------------------------------
            # qpT[h, q] = sum_d Wq[d, h] * queryT[d, q]
            qpT_ps = psum_small.tile([P, 256], f32, tag="small")
            for dc in range(DC):
                nc.tensor.matmul(
                    qpT_ps,
                    lhsT=wq_sb[:, dc],
                    rhs=qT_sb[:, dc],
                    start=(dc == 0),
                    stop=(dc == DC - 1),
                )
            qpT16 = proj.tile([P, L], f16, tag="qpT16")
            nc.vector.tensor_copy(out=qpT16, in_=qpT_ps)

            # kp[k, h] = sum_d keyT[d, k] * Wk[d, h]
            kp16 = proj.tile([P, KT, H], f16, tag="kp16")
            for kt in range(KT):
                kp_ps = psum_small.tile([P, 256], f32, tag="small")
                for dc in range(DC):
                    nc.tensor.matmul(
                        kp_ps[:, :H],
                        lhsT=kT_sb[:, dc, kt * P:(kt + 1) * P],
                        rhs=wk_sb[:, dc],
                        start=(dc == 0),
                        stop=(dc == DC - 1),
                    )
                nc.vector.tensor_copy(out=kp16[:, kt, :], in_=kp_ps[:, :H])

            # --- score accumulation ---------------------------------------
            acc = accp.tile([P, KT, L], f16, tag="acc")
            nc.vector.memset(acc, 0.0)

            for hp in range(H // HH):
                ps = psum_big.tile([P, HH, KT, L], f32, tag="ps")
                T = tanhp.tile([P, HH, KT, L], f16, tag="T")
                for hh in range(HH):
                    h = HH * hp + hh
                    # plane[k, (kt, q)] = kp[kt*128+k, h]  (identity pass)
                    nc.tensor.matmul(
                        ps[:, hh],
                        lhsT=ident16,
                        rhs=kp16[:, :, h].unsqueeze(2).broadcast_to([P, KT, L]),
                        start=True,
                        stop=False,
                    )
                    # plane[k, (kt, q)] += qpT[h, q]  (indicator pass)
                    nc.tensor.matmul(
                        ps[:, hh],
                        lhsT=t128[:, :, h],
                        rhs=qpT16.unsqueeze(1).broadcast_to([P, KT, L]),
                        start=False,
                        stop=True,
                    )
                nc.scalar.activation(out=T, in_=ps, func=AF.Tanh)
                for hh in range(HH):
                    h = HH * hp + hh
                    nc.vector.scalar_tensor_tensor(
                        out=acc,
                        in0=T[:, hh],
                        scalar=vb[:, h:h + 1],
                        in1=acc,
                        op0=OP.mult,
                        op1=OP.add,
                    )

            # --- softmax over q (free dim); scores bounded, skip max -------
            e = soft.tile([P, KT, L], f32, tag="e")
            sums = soft.tile([P, KT], f32, tag="sums")
            rec = soft.tile([P, KT], f32, tag="rec")
            for kt in range(KT):
                nc.scalar.activation(
                    out=e[:, kt],
                    in_=acc[:, kt],
                    func=AF.Exp,
                    accum_out=sums[:, kt:kt + 1],
                )
            nc.vector.reciprocal(rec, sums)
            for kt in range(KT):
                nc.vector.tensor_scalar_mul(e[:, kt], e[:, kt], rec[:, kt:kt + 1])
                nc.sync.dma_start(
                    out=attn_d[b, kt * P:(kt + 1) * P, :], in_=e[:, kt]
                )

            # --- context = attention @ value ------------------------------
            attnT = outp.tile([P, QT, L], f32, tag="attnT")
            for kt in range(KT):
                for qc in range(QT):
                    psT = psum_small.tile([P, 256], f32, tag="small")
                    nc.tensor.transpose(
                        psT[:, :P], e[:, kt, qc * P:(qc + 1) * P], ident32
                    )
                    nc.vector.tensor_copy(
                        out=attnT[:, qc, kt * P:(kt + 1) * P], in_=psT[:, :P]
                    )
            for kt in range(KT):
                ctx_ps = psum_small.tile([P, 256], f32, tag="small")
                for qc in range(QT):
                    nc.tensor.matmul(
                        ctx_ps[:, :VD],
                        lhsT=attnT[:, qc, kt * P:(kt + 1) * P],
                        rhs=val_sb[:, qc],
                        start=(qc == 0),
                        stop=(qc == QT - 1),
                    )
                ctx_sb = outp.tile([P, VD], f32, tag="ctx_sb")
                nc.vector.tensor_copy(out=ctx_sb, in_=ctx_ps[:, :VD])
                nc.sync.dma_start(out=ctx_d[b, kt * P:(kt + 1) * P, :], in_=ctx_sb)

    nc.compile()
    return nc


def _get_nc():
    if "nc" not in _cache:
        _cache["nc"] = _build()
    return _cache["nc"]


def run_sharded(inputs, trace=False, trace_cores=None):
    from concourse.bass_utils import run_bass_kernel_spmd

    nc = _get_nc()
    q = np.ascontiguousarray(np.asarray(inputs["query"]), dtype=np.float32)
    k = np.ascontiguousarray(np.asarray(inputs["key"]), dtype=np.float32)
    val = np.ascontiguousarray(np.asarray(inputs["value"]), dtype=np.float32)
    wq = np.ascontiguousarray(np.asarray(inputs["Wq"]), dtype=np.float32)
    wk = np.ascontiguousarray(np.asarray(inputs["Wk"]), dtype=np.float32)
    vv = np.ascontiguousarray(np.asarray(inputs["v"]), dtype=np.float32)

    in_maps = []
    for c in range(N_CORES):
        sl = slice(c * B_LOC, (c + 1) * B_LOC)
        in_maps.append(
            {
                "query": np.ascontiguousarray(q[sl]),
                "key": np.ascontiguousarray(k[sl]),
                "value": np.ascontiguousarray(val[sl]),
                "Wq": wq,
                "Wk": wk,
                "v": vv,
            }
        )
    kwargs = {}
    if trace_cores is not None:
        kwargs["trace_cores"] = trace_cores
    res = run_bass_kernel_spmd(
        nc, in_maps, core_ids=list(range(N_CORES)), trace=trace, **kwargs
    )
    context = np.concatenate([r["context"] for r in res.results], axis=0)
    attention = np.concatenate([r["attention"] for r in res.results], axis=0)
    return (context, attention), res


def kernel(**inputs):
    (context, attention), _ = run_sharded(inputs, trace=False)
    return context, attention


if __name__ == "__main__":
    nc = _build()
    print("build + compile OK")


# revision 6
# speedup vs baseline: 1.1492x; 1.1492x over previous
# Additive (Bahdanau) attention kernel for Trainium2, data-parallel over batch
# on 8 NeuronCores.
#
# Per core (2 batches):
#   qp = query @ Wq            [256q, 128h]   (kept transposed: qpT [h, q])
#   kp = key @ Wk              [256k, 128h]   (kept natural:    kp  [k, h])
#   score[k, q] = sum_h v[h] * tanh(qp[q, h] + kp[k, h])
#   attention = softmax_q(score);  context = attention @ value
#
# The [k, q] plane for each h is built on the TensorEngine as two matmuls into
# PSUM: an identity pass that broadcasts kp[:, h] along q, plus an indicator
# pass that broadcasts qpT[h, :] along k. ScalarE applies tanh (fp16 out), and
# VectorE folds planes into the score accumulator with one fused
# (T * v_h) + acc op per plane. Softmax runs per k-tile with Exp+accum_out
# (scores are bounded by sum|v| <= 6.5, so no max subtraction is needed).

import numpy as np

B_TOTAL = 16
N_CORES = 8
B_LOC = B_TOTAL // N_CORES
L = 256      # l_q == l_k
D = 256      # q_dim == k_dim
H = 128      # attention dim
VD = 128     # value dim
P = 128      # partitions

_cache = {}


def _build():
    from contextlib import ExitStack

    import concourse.bacc as bacc
    import concourse.mybir as mybir
    import concourse.tile as tile
    
    f32 = mybir.dt.float32
    f16 = mybir.dt.float16
    AF = mybir.ActivationFunctionType
    OP = mybir.AluOpType

    nc = bacc.Bacc("TRN2", target_bir_lowering=False, debug=False)

    q_d = nc.dram_tensor("query", [B_LOC, L, D], f32, kind="ExternalInput")
    k_d = nc.dram_tensor("key", [B_LOC, L, D], f32, kind="ExternalInput")
    val_d = nc.dram_tensor("value", [B_LOC, L, VD], f32, kind="ExternalInput")
    wq_d = nc.dram_tensor("Wq", [D, H], f32, kind="ExternalInput")
    wk_d = nc.dram_tensor("Wk", [D, H], f32, kind="ExternalInput")
    vv_d = nc.dram_tensor("v", [H], f32, kind="ExternalInput")
    ctx_d = nc.dram_tensor("context", [B_LOC, L, VD], f32, kind="ExternalOutput")
    attn_d = nc.dram_tensor("attention", [B_LOC, L, L], f32, kind="ExternalOutput")

    KT = L // P   # k tiles (2)
    QT = L // P   # q tiles (2)
    DC = D // P   # d chunks (2)
    HH = 2        # h planes per PSUM tile (psum tile = HH banks)

    with tile.TileContext(nc) as tc, ExitStack() as ctx:
        singles = ctx.enter_context(tc.tile_pool(name="singles", bufs=1))
        io = ctx.enter_context(tc.tile_pool(name="io", bufs=2))
        tr = ctx.enter_context(tc.tile_pool(name="tr", bufs=2))
        proj = ctx.enter_context(tc.tile_pool(name="proj", bufs=2))
        tanhp = ctx.enter_context(tc.tile_pool(name="tanhp", bufs=3))
        accp = ctx.enter_context(tc.tile_pool(name="accp", bufs=2))
        soft = ctx.enter_context(tc.tile_pool(name="soft", bufs=2))
        outp = ctx.enter_context(tc.tile_pool(name="outp", bufs=2))
        psum_big = ctx.enter_context(tc.tile_pool(name="psum_big", bufs=2, space="PSUM"))
        psum_small = ctx.enter_context(tc.tile_pool(name="psum_small", bufs=2, space="PSUM"))

        # --- constants -----------------------------------------------------
        # delta[p, m] = p - m; one-hot identities via is_equal (much faster
        # than gpsimd affine_select for the big T128 constant below).
        delta = singles.tile([P, P], mybir.dt.int32, tag="delta")
        nc.gpsimd.iota(delta, pattern=[[-1, P]], base=0, channel_multiplier=1)
        ident32 = singles.tile([P, P], f32, tag="ident32")
        nc.vector.tensor_scalar(
            out=ident32, in0=delta, scalar1=0, scalar2=None, op0=OP.is_equal
        )
        ident16 = singles.tile([P, P], f16, tag="ident16")
        nc.vector.tensor_copy(out=ident16, in_=ident32)

        # T128[p, k, h] = 1.0 iff p == h. lhsT slice T128[:, :, h] is a K=128
        # stationary whose row h is all-ones: the matmul broadcasts moving
        # row h (a qpT row) to all 128 output partitions while keeping the
        # full PE array engaged (K=32 variants starve the HAM clock governor).
        t128 = singles.tile([P, P, P], f16, tag="t128")
        nc.vector.tensor_copy(
            out=t128, in_=ident16.unsqueeze(1).broadcast_to([P, P, P])
        )

        vb = singles.tile([P, H], f32, tag="vb")  # v broadcast across partitions
        nc.sync.dma_start(out=vb, in_=vv_d[:].unsqueeze(0).broadcast_to([P, H]))

        # VI[p, h, k] = v[h] * (p == k): lhsT slice VI[:, h, :] is the
        # scaled identity; an accumulating matmul with it folds v_h * T_h
        # into the score PSUM bank (fp32) without touching VectorE.
        vi = singles.tile([P, H, P], f16, tag="vi")
        for h in range(H):
            nc.vector.tensor_scalar_mul(
                out=vi[:, h, :], in0=ident16, scalar1=vb[:, h:h + 1]
            )

        wq_sb = singles.tile([P, DC, H], f32, tag="wq")
        nc.sync.dma_start(out=wq_sb, in_=wq_d[:].rearrange("(c p) h -> p c h", p=P))
        wk_sb = singles.tile([P, DC, H], f32, tag="wk")
        nc.sync.dma_start(out=wk_sb, in_=wk_d[:].rearrange("(c p) h -> p c h", p=P))

        for b in range(B_LOC):
            # --- load ------------------------------------------------------
            q_sb = io.tile([P, QT, D], f32, tag="q_sb")
            nc.sync.dma_start(out=q_sb, in_=q_d[b].rearrange("(t p) d -> p t d", p=P))
            k_sb = io.tile([P, KT, D], f32, tag="k_sb")
            nc.sync.dma_start(out=k_sb, in_=k_d[b].rearrange("(t p) d -> p t d", p=P))
            val_sb = io.tile([P, QT, VD], f32, tag="val_sb")
            nc.sync.dma_start(out=val_sb, in_=val_d[b].rearrange("(t p) v -> p t v", p=P))

            # --- transpose query/key (PE transpose, fp32) ------------------
            qT_sb = tr.tile([P, DC, L], f32, tag="qT")
            kT_sb = tr.tile([P, DC, L], f32, tag="kT")
            for src, dst in ((q_sb, qT_sb), (k_sb, kT_sb)):
                for t in range(QT):
                    for dc in range(DC):
                        psT = psum_small.tile([P, 256], f32, tag="small")
                        nc.tensor.transpose(
                            psT[:, :P], src[:, t, dc * P:(dc + 1) * P], ident32
                        )
                        nc.vector.tensor_copy(
                            out=dst[:, dc, t * P:(t + 1) * P], in_=psT[:, :P]
                        )

            # --- projections ----------------------------------------------
            # qpT[h, q] = sum_d Wq[d, h] * queryT[d, q]
            qpT_ps = psum_small.tile([P, 256], f32, tag="small")
            for dc in range(DC):
                nc.tensor.matmul(
                    qpT_ps,
                    lhsT=wq_sb[:, dc],
                    rhs=qT_sb[:, dc],
                    start=(dc == 0),
                    stop=(dc == DC - 1),
                )
            qpT16 = proj.tile([P, L], f16, tag="qpT16")
            nc.vector.tensor_copy(out=qpT16, in_=qpT_ps)

            # kp[k, h] = sum_d keyT[d, k] * Wk[d, h]
            kp16 = proj.tile([P, KT, H], f16, tag="kp16")
            for kt in range(KT):
                kp_ps = psum_small.tile([P, 256], f32, tag="small")
                for dc in range(DC):
                    nc.tensor.matmul(
                        kp_ps[:, :H],
                        lhsT=kT_sb[:, dc, kt * P:(kt + 1) * P],
                        rhs=wk_sb[:, dc],
                        start=(dc == 0),
                        stop=(dc == DC - 1),
                    )
                nc.vector.tensor_copy(out=kp16[:, kt, :], in_=kp_ps[:, :H])

            # --- score accumulation ---------------------------------------
            # Even-h planes fold into score_ps on the TensorEngine (scaled
            # identity, fp32 PSUM accumulate); odd-h planes fold into acc on
            # VectorE. score = score_ps + acc at the end.
            acc = accp.tile([P, KT, L], f16, tag="acc")
            nc.vector.memset(acc, 0.0)
            score_ps = psum_small.tile([P, KT, L], f32, tag="score")

            for hp in range(H // HH):
                ps = psum_big.tile([P, HH, KT, L], f32, tag="ps")
                T = tanhp.tile([P, HH, KT, L], f16, tag="T")
                for hh in range(HH):
                    h = HH * hp + hh
                    # plane[k, (kt, q)] = kp[kt*128+k, h]  (identity pass)
                    nc.tensor.matmul(
                        ps[:, hh],
                        lhsT=ident16,
                        rhs=kp16[:, :, h].unsqueeze(2).broadcast_to([P, KT, L]),
                        start=True,
                        stop=False,
                    )
                    # plane[k, (kt, q)] += qpT[h, q]  (indicator pass)
                    nc.tensor.matmul(
                        ps[:, hh],
                        lhsT=t128[:, :, h],
                        rhs=qpT16.unsqueeze(1).broadcast_to([P, KT, L]),
                        start=False,
                        stop=True,
                    )
                nc.scalar.activation(out=T, in_=ps, func=AF.Tanh)
                for hh in range(HH):
                    h = HH * hp + hh
                    if h % 2 == 0:
                        nc.tensor.matmul(
                            score_ps,
                            lhsT=vi[:, h, :],
                            rhs=T[:, hh],
                            start=(h == 0),
                            stop=(h == H - 2),
                        )
                    else:
                        nc.vector.scalar_tensor_tensor(
                            out=acc,
                            in0=T[:, hh],
                            scalar=vb[:, h:h + 1],
                            in1=acc,
                            op0=OP.mult,
                            op1=OP.add,
                        )

            scoref = soft.tile([P, KT, L], f32, tag="scoref")
            nc.vector.tensor_tensor(
                out=scoref, in0=score_ps, in1=acc, op=OP.add
            )

            # --- softmax over q (free dim); scores bounded, skip max -------
            e = soft.tile([P, KT, L], f32, tag="e")
            sums = soft.tile([P, KT], f32, tag="sums")
            rec = soft.tile([P, KT], f32, tag="rec")
            for kt in range(KT):
                nc.scalar.activation(
                    out=e[:, kt],
                    in_=scoref[:, kt],
                    func=AF.Exp,
                    accum_out=sums[:, kt:kt + 1],
                )
            nc.vector.reciprocal(rec, sums)
            for kt in range(KT):
                nc.vector.tensor_scalar_mul(e[:, kt], e[:, kt], rec[:, kt:kt + 1])
                nc.sync.dma_start(
                    out=attn_d[b, kt * P:(kt + 1) * P, :], in_=e[:, kt]
                )

            # --- context = attention @ value ------------------------------
            attnT = outp.tile([P, QT, L], f32, tag="attnT")
            for kt in range(KT):
                for qc in range(QT):
                    psT = psum_small.tile([P, 256], f32, tag="small")
                    nc.tensor.transpose(
                        psT[:, :P], e[:, kt, qc * P:(qc + 1) * P], ident32
                    )
                    nc.vector.tensor_copy(
                        out=attnT[:, qc, kt * P:(kt + 1) * P], in_=psT[:, :P]
                    )
            for kt in range(KT):
                ctx_ps = psum_small.tile([P, 256], f32, tag="small")
                for qc in range(QT):
                    nc.tensor.matmul(
                        ctx_ps[:, :VD],
                        lhsT=attnT[:, qc, kt * P:(kt + 1) * P],
                        rhs=val_sb[:, qc],
                        start=(qc == 0),
                        stop=(qc == QT - 1),
                    )
                ctx_sb = outp.tile([P, VD], f32, tag="ctx_sb")
                nc.vector.tensor_copy(out=ctx_sb, in_=ctx_ps[:, :VD])
                nc.sync.dma_start(out=ctx_d[b, kt * P:(kt + 1) * P, :], in_=ctx_sb)

    nc.compile()
    return nc


def _get_nc():
    if "nc" not in _cache:
        _cache["nc"] = _build()
    return _cache["nc"]


def run_sharded(inputs, trace=False, trace_cores=None):
    from concourse.bass_utils import run_bass_kernel_spmd

    nc = _get_nc()
    q = np.ascontiguousarray(np.asarray(inputs["query"]), dtype=np.float32)
    k = np.ascontiguousarray(np.asarray(inputs["key"]), dtype=np.float32)
    val = np.ascontiguousarray(np.asarray(inputs["value"]), dtype=np.float32)
    wq = np.ascontiguousarray(np.asarray(inputs["Wq"]), dtype=np.float32)
    wk = np.ascontiguousarray(np.asarray(inputs["Wk"]), dtype=np.float32)
    vv = np.ascontiguousarray(np.asarray(inputs["v"]), dtype=np.float32)

    in_maps = []
    for c in range(N_CORES):
        sl = slice(c * B_LOC, (c + 1) * B_LOC)
        in_maps.append(
            {
                "query": np.ascontiguousarray(q[sl]),
                "key": np.ascontiguousarray(k[sl]),
                "value": np.ascontiguousarray(val[sl]),
                "Wq": wq,
                "Wk": wk,
                "v": vv,
            }
        )
    kwargs = {}
    if trace_cores is not None:
        kwargs["trace_cores"] = trace_cores
    res = run_bass_kernel_spmd(
        nc, in_maps, core_ids=list(range(N_CORES)), trace=trace, **kwargs
    )
    context = np.concatenate([r["context"] for r in res.results], axis=0)
    attention = np.concatenate([r["attention"] for r in res.results], axis=0)
    return (context, attention), res


def kernel(**inputs):
    (context, attention), _ = run_sharded(inputs, trace=False)
    return context, attention


if __name__ == "__main__":
    nc = _build()
    print("build + compile OK")


# revision 7
# speedup vs baseline: 1.4064x; 1.2238x over previous
# Additive (Bahdanau) attention kernel for Trainium2, data-parallel over batch
# on 8 NeuronCores.
#
# Per core (2 batches):
#   qp = query @ Wq            [256q, 128h]   (kept transposed: qpT [h, q])
#   kp = key @ Wk              [256k, 128h]   (kept natural:    kp  [k, h])
#   score[k, q] = sum_h v[h] * tanh(qp[q, h] + kp[k, h])
#   attention = softmax_q(score);  context = attention @ value
#
# The [k, q] plane for each h is built on the TensorEngine as two K=128
# matmuls into PSUM: an identity pass that broadcasts kp[:, h] along q, plus a
# one-hot-row pass (T128) that broadcasts qpT[h, :] along k. ScalarE applies
# tanh (fp16 out). The v-weighted reduction over h is split: even-h planes
# fold into a score PSUM bank on the TensorEngine via scaled-identity
# accumulating matmuls (VI), odd-h planes fold into two fp16 accumulators on
# VectorE (two chains so pipe drains overlap). Softmax runs per k-tile with
# Exp+accum_out (scores are bounded by sum|v| <= 6.5, so no max subtraction).

import numpy as np

B_TOTAL = 16
N_CORES = 8
B_LOC = B_TOTAL // N_CORES
L = 256      # l_q == l_k
D = 256      # q_dim == k_dim
H = 128      # attention dim
VD = 128     # value dim
P = 128      # partitions

_cache = {}


def _build():
    from contextlib import ExitStack

    import concourse.bacc as bacc
    import concourse.mybir as mybir
    import concourse.tile as tile

    f32 = mybir.dt.float32
    f16 = mybir.dt.float16
    AF = mybir.ActivationFunctionType
    OP = mybir.AluOpType

    nc = bacc.Bacc("TRN2", target_bir_lowering=False, debug=False)

    q_d = nc.dram_tensor("query", [B_LOC, L, D], f32, kind="ExternalInput")
    k_d = nc.dram_tensor("key", [B_LOC, L, D], f32, kind="ExternalInput")
    val_d = nc.dram_tensor("value", [B_LOC, L, VD], f32, kind="ExternalInput")
    wq_d = nc.dram_tensor("Wq", [D, H], f32, kind="ExternalInput")
    wk_d = nc.dram_tensor("Wk", [D, H], f32, kind="ExternalInput")
    vv_d = nc.dram_tensor("v", [H], f32, kind="ExternalInput")
    ctx_d = nc.dram_tensor("context", [B_LOC, L, VD], f32, kind="ExternalOutput")
    attn_d = nc.dram_tensor("attention", [B_LOC, L, L], f32, kind="ExternalOutput")

    KT = L // P   # k tiles (2)
    QT = L // P   # q tiles (2)
    DC = D // P   # d chunks (2)
    HH = 2        # h planes per big PSUM tile (= HH banks)

    with tile.TileContext(nc) as tc, ExitStack() as ctx:
        singles = ctx.enter_context(tc.tile_pool(name="singles", bufs=1))
        io = ctx.enter_context(tc.tile_pool(name="io", bufs=2))
        tr = ctx.enter_context(tc.tile_pool(name="tr", bufs=2))
        proj = ctx.enter_context(tc.tile_pool(name="proj", bufs=2))
        tanhp = ctx.enter_context(tc.tile_pool(name="tanhp", bufs=3))
        accp = ctx.enter_context(tc.tile_pool(name="accp", bufs=2))
        soft = ctx.enter_context(tc.tile_pool(name="soft", bufs=2))
        outp = ctx.enter_context(tc.tile_pool(name="outp", bufs=2))
        # 6 banks of plane tiles (shared with prologue/epilogue via tag "ps")
        # + 2 banks of per-batch score accumulators = all 8 PSUM banks.
        psum_big = ctx.enter_context(tc.tile_pool(name="psum_big", bufs=3, space="PSUM"))
        psum_sc = ctx.enter_context(tc.tile_pool(name="psum_sc", bufs=2, space="PSUM"))

        # --- constants -----------------------------------------------------
        # delta[p, m] = p - m; identities via is_equal.
        delta = singles.tile([P, P], mybir.dt.int32, tag="delta")
        nc.gpsimd.iota(delta, pattern=[[-1, P]], base=0, channel_multiplier=1)
        ident32 = singles.tile([P, P], f32, tag="ident32")
        nc.vector.tensor_scalar(
            out=ident32, in0=delta, scalar1=0, scalar2=None, op0=OP.is_equal
        )
        ident16 = singles.tile([P, P], f16, tag="ident16")
        nc.vector.tensor_copy(out=ident16, in_=ident32)

        # T128[p, h, k] = 1.0 iff p == h: lhsT slice T128[:, h, :] (contiguous,
        # fast weight load) is a K=128 stationary whose row h is all-ones; the
        # matmul broadcasts moving row h (a qpT row) to all 128 output
        # partitions while keeping the full PE array busy (K=32 variants
        # starve the HAM clock governor). Built on otherwise-idle GpSimd.
        t128 = singles.tile([P, P, P], f16, tag="t128")
        nc.gpsimd.memset(t128, 0.0)
        nc.gpsimd.affine_select(
            out=t128,
            in_=t128,
            compare_op=OP.not_equal,
            fill=1.0,
            base=0,
            pattern=[[-1, P], [0, P]],
            channel_multiplier=1,
        )

        vb = singles.tile([P, H], f32, tag="vb")  # v broadcast across partitions
        nc.sync.dma_start(out=vb, in_=vv_d[:].unsqueeze(0).broadcast_to([P, H]))
        vb16 = singles.tile([P, H], f16, tag="vb16")
        nc.vector.tensor_copy(out=vb16, in_=vb)

        # VI[p, h, k] = v[h] * (p == k): lhsT slice VI[:, h, :] is a scaled
        # identity; an accumulating matmul with it folds v_h * T_h into the
        # score PSUM bank (fp32) without touching VectorE.
        vi = singles.tile([P, H, P], f16, tag="vi")
        nc.vector.tensor_tensor(
            out=vi,
            in0=ident16.unsqueeze(1).broadcast_to([P, H, P]),
            in1=vb16.unsqueeze(2).broadcast_to([P, H, P]),
            op=OP.mult,
        )

        wq_sb = singles.tile([P, DC, H], f32, tag="wq")
        nc.sync.dma_start(out=wq_sb, in_=wq_d[:].rearrange("(c p) h -> p c h", p=P))
        wk_sb = singles.tile([P, DC, H], f32, tag="wk")
        nc.sync.dma_start(out=wk_sb, in_=wk_d[:].rearrange("(c p) h -> p c h", p=P))

        for b in range(B_LOC):
            # --- load ------------------------------------------------------
            q_sb = io.tile([P, QT, D], f32, tag="q_sb")
            nc.sync.dma_start(out=q_sb, in_=q_d[b].rearrange("(t p) d -> p t d", p=P))
            k_sb = io.tile([P, KT, D], f32, tag="k_sb")
            nc.sync.dma_start(out=k_sb, in_=k_d[b].rearrange("(t p) d -> p t d", p=P))
            val_sb = io.tile([P, QT, VD], f32, tag="val_sb")
            nc.sync.dma_start(out=val_sb, in_=val_d[b].rearrange("(t p) v -> p t v", p=P))

            # --- transpose query/key (PE transpose, fp32) ------------------
            qT_sb = tr.tile([P, DC, L], f32, tag="qT")
            kT_sb = tr.tile([P, DC, L], f32, tag="kT")
            for src, dst in ((q_sb, qT_sb), (k_sb, kT_sb)):
                for t in range(QT):
                    for dc in range(DC):
                        psT = psum_big.tile([P, HH, KT, L], f32, tag="ps")
                        nc.tensor.transpose(
                            psT[:, 0, 0, :P], src[:, t, dc * P:(dc + 1) * P], ident32
                        )
                        nc.vector.tensor_copy(
                            out=dst[:, dc, t * P:(t + 1) * P], in_=psT[:, 0, 0, :P]
                        )

            # --- projections ----------------------------------------------
            # qpT[h, q] = sum_d Wq[d, h] * queryT[d, q]
            qpT_ps = psum_big.tile([P, HH, KT, L], f32, tag="ps")
            for dc in range(DC):
                nc.tensor.matmul(
                    qpT_ps[:, 0, 0, :],
                    lhsT=wq_sb[:, dc],
                    rhs=qT_sb[:, dc],
                    start=(dc == 0),
                    stop=(dc == DC - 1),
                )
            qpT16 = proj.tile([P, L], f16, tag="qpT16")
            nc.vector.tensor_copy(out=qpT16, in_=qpT_ps[:, 0, 0, :])

            # kp[k, h] = sum_d keyT[d, k] * Wk[d, h]
            kp16 = proj.tile([P, KT, H], f16, tag="kp16")
            for kt in range(KT):
                kp_ps = psum_big.tile([P, HH, KT, L], f32, tag="ps")
                for dc in range(DC):
                    nc.tensor.matmul(
                        kp_ps[:, 0, 0, :H],
                        lhsT=kT_sb[:, dc, kt * P:(kt + 1) * P],
                        rhs=wk_sb[:, dc],
                        start=(dc == 0),
                        stop=(dc == DC - 1),
                    )
                nc.vector.tensor_copy(out=kp16[:, kt, :], in_=kp_ps[:, 0, 0, :H])

            # --- score accumulation ---------------------------------------
            # Even-h planes fold into score_ps on the TensorEngine (VI,
            # fp32 PSUM accumulate); odd-h planes alternate between two fp16
            # VectorE accumulators. score = score_ps + acc_a + acc_b.
            acc_a = accp.tile([P, KT, L], f16, tag="acc_a")
            nc.vector.memset(acc_a, 0.0)
            acc_b = accp.tile([P, KT, L], f16, tag="acc_b")
            nc.vector.memset(acc_b, 0.0)
            score_ps = psum_sc.tile([P, KT, L], f32, tag="score")

            for hp in range(H // HH):
                ps = psum_big.tile([P, HH, KT, L], f32, tag="ps")
                T = tanhp.tile([P, HH, KT, L], f16, tag="T")
                for hh in range(HH):
                    h = HH * hp + hh
                    # plane[k, (kt, q)] = kp[kt*128+k, h]  (identity pass)
                    nc.tensor.matmul(
                        ps[:, hh],
                        lhsT=ident16,
                        rhs=kp16[:, :, h].unsqueeze(2).broadcast_to([P, KT, L]),
                        start=True,
                        stop=False,
                    )
                    # plane[k, (kt, q)] += qpT[h, q]  (one-hot row pass)
                    nc.tensor.matmul(
                        ps[:, hh],
                        lhsT=t128[:, h, :],
                        rhs=qpT16.unsqueeze(1).broadcast_to([P, KT, L]),
                        start=False,
                        stop=True,
                    )
                nc.scalar.activation(out=T, in_=ps, func=AF.Tanh)
                for hh in range(HH):
                    h = HH * hp + hh
                    if h % 2 == 0:
                        nc.tensor.matmul(
                            score_ps,
                            lhsT=vi[:, h, :],
                            rhs=T[:, hh],
                            start=(h == 0),
                            stop=(h == H - 2),
                        )
                    else:
                        acc = acc_a if (h % 4 == 1) else acc_b
                        nc.vector.scalar_tensor_tensor(
                            out=acc,
                            in0=T[:, hh],
                            scalar=vb[:, h:h + 1],
                            in1=acc,
                            op0=OP.mult,
                            op1=OP.add,
                        )

            scoref = soft.tile([P, KT, L], f32, tag="scoref")
            nc.vector.scalar_tensor_tensor(
                out=scoref, in0=acc_a, scalar=1.0, in1=acc_b,
                op0=OP.mult, op1=OP.add,
            )
            nc.vector.tensor_tensor(
                out=scoref, in0=scoref, in1=score_ps, op=OP.add
            )

            # --- softmax over q (free dim); scores bounded, skip max -------
            e = soft.tile([P, KT, L], f32, tag="e")
            sums = soft.tile([P, KT], f32, tag="sums")
            rec = soft.tile([P, KT], f32, tag="rec")
            for kt in range(KT):
                nc.scalar.activation(
                    out=e[:, kt],
                    in_=scoref[:, kt],
                    func=AF.Exp,
                    accum_out=sums[:, kt:kt + 1],
                )
            nc.vector.reciprocal(rec, sums)
            for kt in range(KT):
                nc.vector.tensor_scalar_mul(e[:, kt], e[:, kt], rec[:, kt:kt + 1])
                nc.sync.dma_start(
                    out=attn_d[b, kt * P:(kt + 1) * P, :], in_=e[:, kt]
                )

            # --- context = attention @ value ------------------------------
            attnT = outp.tile([P, QT, L], f32, tag="attnT")
            for kt in range(KT):
                for qc in range(QT):
                    psT = psum_big.tile([P, HH, KT, L], f32, tag="ps")
                    nc.tensor.transpose(
                        psT[:, 0, 0, :P], e[:, kt, qc * P:(qc + 1) * P], ident32
                    )
                    nc.vector.tensor_copy(
                        out=attnT[:, qc, kt * P:(kt + 1) * P], in_=psT[:, 0, 0, :P]
                    )
            for kt in range(KT):
                ctx_ps = psum_big.tile([P, HH, KT, L], f32, tag="ps")
                for qc in range(QT):
                    nc.tensor.matmul(
                        ctx_ps[:, 0, 0, :VD],
                        lhsT=attnT[:, qc, kt * P:(kt + 1) * P],
                        rhs=val_sb[:, qc],
                        start=(qc == 0),
                        stop=(qc == QT - 1),
                    )
                ctx_sb = outp.tile([P, VD], f32, tag="ctx_sb")
                nc.vector.tensor_copy(out=ctx_sb, in_=ctx_ps[:, 0, 0, :VD])
                nc.sync.dma_start(out=ctx_d[b, kt * P:(kt + 1) * P, :], in_=ctx_sb)

    nc.compile()
    return nc


def _get_nc():
    if "nc" not in _cache:
        _cache["nc"] = _build()
    return _cache["nc"]


def run_sharded(inputs, trace=False, trace_cores=None):
    from concourse.bass_utils import run_bass_kernel_spmd

    nc = _get_nc()
    q = np.ascontiguousarray(np.asarray(inputs["query"]), dtype=np.float32)
    k = np.ascontiguousarray(np.asarray(inputs["key"]), dtype=np.float32)
    val = np.ascontiguousarray(np.asarray(inputs["value"]), dtype=np.float32)
    wq = np.ascontiguousarray(np.asarray(inputs["Wq"]), dtype=np.float32)
    wk = np.ascontiguousarray(np.asarray(inputs["Wk"]), dtype=np.float32)
    vv = np.ascontiguousarray(np.asarray(inputs["v"]), dtype=np.float32)

    in_maps = []
    for c in range(N_CORES):
        sl = slice(c * B_LOC, (c + 1) * B_LOC)
        in_maps.append(
            {
                "query": np.ascontiguousarray(q[sl]),
                "key": np.ascontiguousarray(k[sl]),
                "value": np.ascontiguousarray(val[sl]),
                "Wq": wq,
                "Wk": wk,
                "v": vv,
            }
        )
    kwargs = {}
    if trace_cores is not None:
        kwargs["trace_cores"] = trace_cores
    res = run_bass_kernel_spmd(
        nc, in_maps, core_ids=list(range(N_CORES)), trace=trace, **kwargs
    )
    context = np.concatenate([r["context"] for r in res.results], axis=0)
    attention = np.concatenate([r["attention"] for r in res.results], axis=0)
    return (context, attention), res


def kernel(**inputs):
    (context, attention), _ = run_sharded(inputs, trace=False)
    return context, attention


if __name__ == "__main__":
    nc = _build()
    print("build + compile OK")


# revision 9
# speedup vs baseline: 1.5354x; 1.0917x over previous
# Additive (Bahdanau) attention kernel for Trainium2, data-parallel over batch
# on 8 NeuronCores.
#
# Per core (2 batches):
#   qp = query @ Wq            [256q, 128h]   (kept transposed: qpT [h, q])
#   kp = key @ Wk              [256k, 128h]   (kept natural:    kp  [k, h])
#   score[k, q] = sum_h v[h] * tanh(qp[q, h] + kp[k, h])
#   attention = softmax_q(score);  context = attention @ value
#
# The [k, q] plane for each h is built on the TensorEngine as two K=128
# matmuls into PSUM: an identity pass that broadcasts kp[:, h] along q, plus a
# one-hot-row pass (T128) that broadcasts qpT[h, :] along k. ScalarE applies
# tanh (fp16 out). The v-weighted reduction over h is split: even-h planes
# fold into a score PSUM bank on the TensorEngine via scaled-identity
# accumulating matmuls (VI), odd-h planes fold into two fp16 accumulators on
# VectorE (two chains so pipe drains overlap). Softmax runs per k-tile with
# Exp+accum_out (scores are bounded by sum|v| <= 6.5, so no max subtraction).

import numpy as np

B_TOTAL = 16
N_CORES = 8
B_LOC = B_TOTAL // N_CORES
L = 256      # l_q == l_k
D = 256      # q_dim == k_dim
H = 128      # attention dim
VD = 128     # value dim
P = 128      # partitions

_cache = {}


def _build():
    from contextlib import ExitStack

    import concourse.bacc as bacc
    import concourse.mybir as mybir
    import concourse.tile as tile

    f32 = mybir.dt.float32
    f16 = mybir.dt.float16
    AF = mybir.ActivationFunctionType
    OP = mybir.AluOpType

    nc = bacc.Bacc("TRN2", target_bir_lowering=False, debug=False)

    q_d = nc.dram_tensor("query", [B_LOC, L, D], f32, kind="ExternalInput")
    k_d = nc.dram_tensor("key", [B_LOC, L, D], f32, kind="ExternalInput")
    val_d = nc.dram_tensor("value", [B_LOC, L, VD], f32, kind="ExternalInput")
    wq_d = nc.dram_tensor("Wq", [D, H], f32, kind="ExternalInput")
    wk_d = nc.dram_tensor("Wk", [D, H], f32, kind="ExternalInput")
    vv_d = nc.dram_tensor("v", [H], f32, kind="ExternalInput")
    ctx_d = nc.dram_tensor("context", [B_LOC, L, VD], f32, kind="ExternalOutput")
    attn_d = nc.dram_tensor("attention", [B_LOC, L, L], f32, kind="ExternalOutput")

    KT = L // P   # k tiles (2)
    QT = L // P   # q tiles (2)
    DC = D // P   # d chunks (2)
    HH = 2        # h planes per big PSUM tile (= HH banks)

    with tile.TileContext(nc) as tc, ExitStack() as ctx:
        singles = ctx.enter_context(tc.tile_pool(name="singles", bufs=1))
        io = ctx.enter_context(tc.tile_pool(name="io", bufs=2))
        tr = ctx.enter_context(tc.tile_pool(name="tr", bufs=2))
        proj = ctx.enter_context(tc.tile_pool(name="proj", bufs=2))
        tanhp = ctx.enter_context(tc.tile_pool(name="tanhp", bufs=3))
        accp = ctx.enter_context(tc.tile_pool(name="accp", bufs=2))
        soft = ctx.enter_context(tc.tile_pool(name="soft", bufs=2))
        outp = ctx.enter_context(tc.tile_pool(name="outp", bufs=2))
        # 6 banks of plane tiles (shared with prologue/epilogue via tag "ps")
        # + 2 banks of per-batch score accumulators = all 8 PSUM banks.
        psum_big = ctx.enter_context(tc.tile_pool(name="psum_big", bufs=3, space="PSUM"))
        psum_sc = ctx.enter_context(tc.tile_pool(name="psum_sc", bufs=2, space="PSUM"))

        # --- constants -----------------------------------------------------
        def onehot(dst):
            # dst[p, ...free..., m] = 1.0 iff p == m (+base offset per chunk)
            pat = []
            for dim in dst.shape[1:]:
                pat.append([0, dim])
            pat[-1][0] = -1
            nc.gpsimd.memset(dst, 0.0)
            nc.gpsimd.affine_select(
                out=dst, in_=dst, compare_op=OP.not_equal, fill=1.0,
                base=0, pattern=pat, channel_multiplier=1,
            )

        ident32 = singles.tile([P, P], f32, tag="ident32")
        onehot(ident32)
        ident16 = singles.tile([P, P], f16, tag="ident16")
        onehot(ident16)

        # T128[p, h, k] = 1.0 iff p == h: lhsT slice T128[:, h, :] (contiguous,
        # fast weight load) is a K=128 stationary whose row h is all-ones; the
        # matmul broadcasts moving row h (a qpT row) to all 128 output
        # partitions while keeping the full PE array busy (K=32 variants
        # starve the HAM clock governor). Built on otherwise-idle GpSimd in
        # 32-h chunks so only the first chunk gates the main loop.
        t128 = singles.tile([P, P, P], f16, tag="t128")
        for c in range(4):
            chunk = t128[:, 32 * c:32 * (c + 1), :]
            nc.gpsimd.memset(chunk, 0.0)
            nc.gpsimd.affine_select(
                out=chunk, in_=chunk, compare_op=OP.not_equal, fill=1.0,
                base=-32 * c, pattern=[[-1, 32], [0, P]], channel_multiplier=1,
            )

        vb = singles.tile([P, H], f32, tag="vb")  # v broadcast across partitions
        nc.sync.dma_start(out=vb, in_=vv_d[:].unsqueeze(0).broadcast_to([P, H]))
        vb16 = singles.tile([P, H], f16, tag="vb16")
        nc.scalar.copy(out=vb16, in_=vb)

        # VI[p, h, k] = v[h] * (p == k): lhsT slice VI[:, h, :] is a scaled
        # identity; an accumulating matmul with it folds v_h * T_h into the
        # score PSUM bank (fp32) without touching VectorE. Chunked builds.
        vi = singles.tile([P, H, P], f16, tag="vi")
        for c in range(4):
            nc.vector.tensor_tensor(
                out=vi[:, 32 * c:32 * (c + 1), :],
                in0=ident16.unsqueeze(1).broadcast_to([P, 32, P]),
                in1=vb16[:, 32 * c:32 * (c + 1)].unsqueeze(2).broadcast_to([P, 32, P]),
                op=OP.mult,
            )

        wq_sb = singles.tile([P, DC, H], f32, tag="wq")
        nc.sync.dma_start(out=wq_sb, in_=wq_d[:].rearrange("(c p) h -> p c h", p=P))
        wk_sb = singles.tile([P, DC, H], f32, tag="wk")
        nc.sync.dma_start(out=wk_sb, in_=wk_d[:].rearrange("(c p) h -> p c h", p=P))

        for b in range(B_LOC):
            # --- load ------------------------------------------------------
            q_sb = io.tile([P, QT, D], f32, tag="q_sb")
            nc.sync.dma_start(out=q_sb, in_=q_d[b].rearrange("(t p) d -> p t d", p=P))
            k_sb = io.tile([P, KT, D], f32, tag="k_sb")
            nc.sync.dma_start(out=k_sb, in_=k_d[b].rearrange("(t p) d -> p t d", p=P))
            val_sb = io.tile([P, QT, VD], f32, tag="val_sb")
            nc.sync.dma_start(out=val_sb, in_=val_d[b].rearrange("(t p) v -> p t v", p=P))

            # --- transpose query/key (PE transpose, fp32) ------------------
            qT_sb = tr.tile([P, DC, L], f32, tag="qT")
            kT_sb = tr.tile([P, DC, L], f32, tag="kT")
            for src, dst in ((q_sb, qT_sb), (k_sb, kT_sb)):
                for t in range(QT):
                    for dc in range(DC):
                        psT = psum_big.tile([P, HH, KT, L], f32, tag="ps")
                        nc.tensor.transpose(
                            psT[:, 0, 0, :P], src[:, t, dc * P:(dc + 1) * P], ident32
                        )
                        nc.vector.tensor_copy(
                            out=dst[:, dc, t * P:(t + 1) * P], in_=psT[:, 0, 0, :P]
                        )

            # --- projections ----------------------------------------------
            # qpT[h, q] = sum_d Wq[d, h] * queryT[d, q]
            qpT_ps = psum_big.tile([P, HH, KT, L], f32, tag="ps")
            for dc in range(DC):
                nc.tensor.matmul(
                    qpT_ps[:, 0, 0, :],
                    lhsT=wq_sb[:, dc],
                    rhs=qT_sb[:, dc],
                    start=(dc == 0),
                    stop=(dc == DC - 1),
                )
            qpT16 = proj.tile([P, L], f16, tag="qpT16")
            nc.scalar.copy(out=qpT16, in_=qpT_ps[:, 0, 0, :])

            # kp[k, h] = sum_d keyT[d, k] * Wk[d, h]
            kp16 = proj.tile([P, KT, H], f16, tag="kp16")
            for kt in range(KT):
                kp_ps = psum_big.tile([P, HH, KT, L], f32, tag="ps")
                for dc in range(DC):
                    nc.tensor.matmul(
                        kp_ps[:, 0, 0, :H],
                        lhsT=kT_sb[:, dc, kt * P:(kt + 1) * P],
                        rhs=wk_sb[:, dc],
                        start=(dc == 0),
                        stop=(dc == DC - 1),
                    )
                nc.scalar.copy(out=kp16[:, kt, :], in_=kp_ps[:, 0, 0, :H])

            # --- score accumulation ---------------------------------------
            # Even-h planes fold into score_ps on the TensorEngine (VI,
            # fp32 PSUM accumulate); odd-h planes alternate between two fp16
            # VectorE accumulators. score = score_ps + acc_a + acc_b.
            acc_a = accp.tile([P, KT, L], f16, tag="acc_a")
            nc.vector.memset(acc_a, 0.0)
            acc_b = accp.tile([P, KT, L], f16, tag="acc_b")
            nc.vector.memset(acc_b, 0.0)
            score_ps = psum_sc.tile([P, KT, L], f32, tag="score")

            for hp in range(H // HH):
                ps = psum_big.tile([P, HH, KT, L], f32, tag="ps")
                T = tanhp.tile([P, HH, KT, L], f16, tag="T")
                for hh in range(HH):
                    h = HH * hp + hh
                    # plane[k, (kt, q)] = kp[kt*128+k, h]  (identity pass)
                    nc.tensor.matmul(
                        ps[:, hh],
                        lhsT=ident16,
                        rhs=kp16[:, :, h].unsqueeze(2).broadcast_to([P, KT, L]),
                        start=True,
                        stop=False,
                    )
                    # plane[k, (kt, q)] += qpT[h, q]  (one-hot row pass)
                    nc.tensor.matmul(
                        ps[:, hh],
                        lhsT=t128[:, h, :],
                        rhs=qpT16.unsqueeze(1).broadcast_to([P, KT, L]),
                        start=False,
                        stop=True,
                    )
                nc.scalar.activation(out=T, in_=ps, func=AF.Tanh)
                for hh in range(HH):
                    h = HH * hp + hh
                    if h % 2 == 0:
                        nc.tensor.matmul(
                            score_ps,
                            lhsT=vi[:, h, :],
                            rhs=T[:, hh],
                            start=(h == 0),
                            stop=(h == H - 2),
                        )
                    else:
                        acc = acc_a if (h % 4 == 1) else acc_b
                        nc.vector.scalar_tensor_tensor(
                            out=acc,
                            in0=T[:, hh],
                            scalar=vb[:, h:h + 1],
                            in1=acc,
                            op0=OP.mult,
                            op1=OP.add,
                        )

            scoref = soft.tile([P, KT, L], f32, tag="scoref")
            nc.vector.scalar_tensor_tensor(
                out=scoref, in0=acc_a, scalar=1.0, in1=acc_b,
                op0=OP.mult, op1=OP.add,
            )
            nc.vector.tensor_tensor(
                out=scoref, in0=scoref, in1=score_ps, op=OP.add
            )

            # --- softmax over q (free dim); scores bounded, skip max -------
            e = soft.tile([P, KT, L], f32, tag="e")
            sums = soft.tile([P, KT], f32, tag="sums")
            rec = soft.tile([P, KT], f32, tag="rec")
            for kt in range(KT):
                nc.scalar.activation(
                    out=e[:, kt],
                    in_=scoref[:, kt],
                    func=AF.Exp,
                    accum_out=sums[:, kt:kt + 1],
                )
            nc.vector.reciprocal(rec, sums)
            for kt in range(KT):
                nc.vector.tensor_scalar_mul(e[:, kt], e[:, kt], rec[:, kt:kt + 1])
                nc.sync.dma_start(
                    out=attn_d[b, kt * P:(kt + 1) * P, :], in_=e[:, kt]
                )

            # --- context = attention @ value ------------------------------
            attnT = outp.tile([P, QT, L], f32, tag="attnT")
            for kt in range(KT):
                for qc in range(QT):
                    psT = psum_big.tile([P, HH, KT, L], f32, tag="ps")
                    nc.tensor.transpose(
                        psT[:, 0, 0, :P], e[:, kt, qc * P:(qc + 1) * P], ident32
                    )
                    nc.vector.tensor_copy(
                        out=attnT[:, qc, kt * P:(kt + 1) * P], in_=psT[:, 0, 0, :P]
                    )
            for kt in range(KT):
                ctx_ps = psum_big.tile([P, HH, KT, L], f32, tag="ps")
                for qc in range(QT):
                    nc.tensor.matmul(
                        ctx_ps[:, 0, 0, :VD],
                        lhsT=attnT[:, qc, kt * P:(kt + 1) * P],
                        rhs=val_sb[:, qc],
                        start=(qc == 0),
                        stop=(qc == QT - 1),
                    )
                ctx_sb = outp.tile([P, VD], f32, tag="ctx_sb")
                nc.vector.tensor_copy(out=ctx_sb, in_=ctx_ps[:, 0, 0, :VD])
                nc.sync.dma_start(out=ctx_d[b, kt * P:(kt + 1) * P, :], in_=ctx_sb)

    nc.compile()
    return nc


def _get_nc():
    if "nc" not in _cache:
        _cache["nc"] = _build()
    return _cache["nc"]


def run_sharded(inputs, trace=False, trace_cores=None):
    from concourse.bass_utils import run_bass_kernel_spmd

    nc = _get_nc()
    q = np.ascontiguousarray(np.asarray(inputs["query"]), dtype=np.float32)
    k = np.ascontiguousarray(np.asarray(inputs["key"]), dtype=np.float32)
    val = np.ascontiguousarray(np.asarray(inputs["value"]), dtype=np.float32)
    wq = np.ascontiguousarray(np.asarray(inputs["Wq"]), dtype=np.float32)
    wk = np.ascontiguousarray(np.asarray(inputs["Wk"]), dtype=np.float32)
    vv = np.ascontiguousarray(np.asarray(inputs["v"]), dtype=np.float32)

    in_maps = []
    for c in range(N_CORES):
        sl = slice(c * B_LOC, (c + 1) * B_LOC)
        in_maps.append(
            {
                "query": np.ascontiguousarray(q[sl]),
                "key": np.ascontiguousarray(k[sl]),
                "value": np.ascontiguousarray(val[sl]),
                "Wq": wq,
                "Wk": wk,
                "v": vv,
            }
        )
    kwargs = {}
    if trace_cores is not None:
        kwargs["trace_cores"] = trace_cores
    res = run_bass_kernel_spmd(
        nc, in_maps, core_ids=list(range(N_CORES)), trace=trace, **kwargs
    )
    context = np.concatenate([r["context"] for r in res.results], axis=0)
    attention = np.concatenate([r["attention"] for r in res.results], axis=0)
    return (context, attention), res


def kernel(**inputs):
    (context, attention), _ = run_sharded(inputs, trace=False)
    return context, attention


if __name__ == "__main__":
    nc = _build()
    print("build + compile OK")
